# revision 47
# baseline (speedup 1.0000x reference)
"""KoLeo loss kernel for Trainium2 (8 NeuronCores).

Computes -mean(log(||x_i - x_{nn(i)} + eps||)) where x = row-normalized
student_output and nn(i) is the nearest neighbor by max inner product
(diagonal excluded). For unit vectors ||x_i - x_j||^2 = 2 - 2*<x_i,x_j>,
so only the per-row max off-diagonal inner product m_i is needed. Each
core handles a 2048-row block (input rolled so its rows are local 0..2047).

Design (v3):
  - Phase 1 (per 2048-row batch): DMA row tiles, batched squares on ACT +
    one 3D DVE reduce for norms^2, all-DVE rsqrt (quake guess + 2 Newton),
    scale rows to unit length in bf16 with the two 128-wide d-halves split
    (DVE), one DMA-XBAR transpose (sync queue) -> k-major transposed tile,
    then GpSimd casts bf16 -> fp8e4 (max(a,a) tensor_tensor, the one
    elementwise op the Pool engine supports).
  - Phase 2 (per j-group, emitted interleaved two batches behind phase 1):
    fp8 DoubleRow matmuls (K=256 in one pass, N=512 chunks), so half the
    matmul instructions of a bf16 version at the same ~379ns/MM stream
    rate this silicon sustains.
  - Per-unit [128,2048] PSUM drain split ~50/50 between the only two
    PSUM-capable engines: DVE exact reduce_max; ACT exp-accumulate
    (log-sum-exp upper bound of max, beta=256 shift=0.45, adds ~1e-3
    relative loss error, validated offline). Diagonal killed by a -3 mask
    add before either consumer.
  - Host: m = max(dve_max, shift + log(sum exp)/beta) per row, then loss.
    fp8e4 quantization of unit rows adds ~2e-3 relative loss error
    (validated offline; gate is 2e-2).
"""

import numpy as np

import concourse.bass as bass
import concourse.mybir as mybir
import concourse.tile as tile
from concourse import bacc
from concourse import bass_utils

N = 16384
D = 256
NCORES = 8
ROWS = N // NCORES          # 2048 rows per core
ITILES = ROWS // 128        # 16 i-tiles per core
NT = N // 128               # 128 row-tiles of the full matrix
GW = 2048                   # j-group width (4 PSUM banks of fp32)
NGROUPS = N // GW           # 8 j-groups
NB = 16                     # row-tiles per batch (= one j-group)
EPS = 1e-8
BETA = 256.0
SHIFT = 0.45

_CACHE = {}


def _build():
    f32 = mybir.dt.float32
    bf16 = mybir.dt.bfloat16
    fp8 = mybir.dt.float8e4
    i32 = mybir.dt.int32
    AF = mybir.ActivationFunctionType
    ALU = mybir.AluOpType

    nc = bacc.Bacc("TRN2", target_bir_lowering=False, debug=False)
    x = nc.dram_tensor("x", [NGROUPS, NB, 128, D], f32, kind="ExternalInput").ap()
    md_out = nc.dram_tensor("md_out", [128, ITILES * NGROUPS], f32,
                            kind="ExternalOutput").ap()
    ms_out = nc.dram_tensor("ms_out", [128, ITILES * NGROUPS], f32,
                            kind="ExternalOutput").ap()

    with tile.TileContext(nc) as tc:
        with (
            tc.tile_pool(name="singles", bufs=1) as singles,
            tc.tile_pool(name="sbig", bufs=2) as sbig_pool,
            tc.tile_pool(name="sq", bufs=2) as sq_pool,
            tc.tile_pool(name="xsb", bufs=2) as xsb_pool,
            tc.tile_pool(name="xt", bufs=1) as xt_pool,
            tc.tile_pool(name="small", bufs=6) as small,
            tc.tile_pool(name="esc", bufs=2) as esc_pool,
            tc.tile_pool(name="dpsum", bufs=2, space="PSUM") as dpsum,
        ):
            # Diagonal knock-out mask: -3 on the diagonal of a 128x128 block.
            mneg = singles.tile([128, 128], f32, tag="mneg")
            nc.gpsimd.memset(mneg[:], 0.0)
            nc.gpsimd.affine_select(
                out=mneg[:], in_=mneg[:], compare_op=ALU.not_equal,
                fill=-3.0, base=0, pattern=[[-1, 128]], channel_multiplier=1,
            )

            ebias = singles.tile([128, 1], f32, tag="ebias")
            nc.vector.memset(ebias[:], -BETA * SHIFT)

            ss = singles.tile([128, NT], f32, tag="ss")      # norms^2, [p, tile]
            md = singles.tile([128, ITILES * NGROUPS], f32, tag="md")
            ms = singles.tile([128, ITILES * NGROUPS], f32, tag="ms")
            nc.vector.memset(md[:], -3.0)
            nc.vector.memset(ms[:], 0.0)

            # k-major transposed normalized tiles per j-group:
            # xt[g][dd, k*2048 + j] = xnorm[g*2048 + j, k*128 + dd]
            xt = [
                xt_pool.tile([128, 2 * GW], bf16, tag=f"xt{g}", name=f"xt{g}")
                for g in range(NGROUPS)
            ]
            xt8 = [
                xt_pool.tile([128, 2 * GW], fp8, tag=f"xt8_{g}", name=f"xt8_{g}")
                for g in range(NGROUPS)
            ]
            xt8v = [
                xt8[g][:].rearrange("p (k j) -> p k j", k=2)
                for g in range(NGROUPS)
            ]

            # PE warm-up: 32 junk matmuls so the HAM clock-gate is open
            # before the real dot stream arrives.
            wj = singles.tile([128, 1024], bf16, tag="wj")
            nc.vector.memset(wj[:], 0.0)
            wp = dpsum.tile([128, GW], f32, tag="pg")
            for _ in range(48):
                nc.tensor.matmul(
                    wp[:, 0:512], wj[:, 0:128], wj[:, 512:1024],
                    start=True, stop=True,
                )

            def emit_phase1(b, nsplit=1):
                # nsplit>1 processes the batch in tile sub-ranges so the
                # first group's fp8 operand lands sooner (used for batch 0)
                xsb = xsb_pool.tile([128, 2 * NB * 128], bf16, tag="xsb")
                xsbv = xsb[:].rearrange("p (k t c) -> p k t c", k=2, c=128)
                xtv_b = xt[b][:].rearrange("p (s c) -> p s c", c=128)
                xt8v_b = xt8[b][:].rearrange("p (k j) -> p k j", k=2)
                xtbv_b = xt[b][:].rearrange("p (k j) -> p k j", k=2)
                cn = NB // nsplit
                for s in range(nsplit):
                    t0 = s * cn
                    sb = sbig_pool.tile([128, cn, D], f32, tag=f"sb{nsplit}")
                    nc.sync.dma_start(
                        out=sb[:],
                        in_=x[b, t0:t0 + cn].rearrange("t p d -> p t d"),
                    )
                    # norms^2: batched squares (ACT) + one 3D reduce (DVE)
                    sqb = sq_pool.tile([128, cn * D], f32, tag=f"sq{nsplit}")
                    sb_flat = sb[:].rearrange("p t d -> p (t d)")
                    nc.scalar.activation(sqb[:], sb_flat, AF.Square)
                    ssb = ss[:, b * NB + t0:b * NB + t0 + cn]
                    nc.vector.tensor_reduce(
                        ssb, sqb[:].rearrange("p (t d) -> p t d", d=D),
                        axis=mybir.AxisListType.X, op=ALU.add,
                    )

                    # r = rsqrt(ss): quake guess + 2 Newton steps, all DVE
                    sh = small.tile([128, cn], i32, tag="sh")
                    nc.vector.tensor_scalar(
                        out=sh[:], in0=ssb.bitcast(i32), scalar1=1,
                        scalar2=None, op0=ALU.logical_shift_right,
                    )
                    r0 = small.tile([128, cn], i32, tag="r0")
                    nc.vector.tensor_scalar(
                        out=r0[:], in0=sh[:], scalar1=-1, scalar2=0x5F3759DF,
                        op0=ALU.mult, op1=ALU.add,
                    )
                    r = r0[:].bitcast(f32)
                    for _ in range(2):
                        t1 = small.tile([128, cn], f32, tag="t1")
                        nc.vector.tensor_mul(t1[:], r, r)
                        nc.vector.tensor_mul(t1[:], t1[:], ssb)
                        nc.vector.tensor_scalar(
                            out=t1[:], in0=t1[:], scalar1=-0.5, scalar2=1.5,
                            op0=ALU.mult, op1=ALU.add,
                        )
                        r2 = small.tile([128, cn], f32, tag="rr")
                        nc.vector.tensor_mul(r2[:], r, t1[:])
                        r = r2[:]

                    # scale rows to unit norm -> bf16, d-halves split k-major
                    for t in range(cn):
                        nc.vector.tensor_scalar_mul(
                            xsbv[:, :, t0 + t, :],
                            sb[:, t, :].rearrange("p (k c) -> p k c", k=2),
                            r[:, t:t + 1],
                        )
                    if nsplit == 1:
                        # whole batch: one XBAR transpose + one cast DMA
                        nc.sync.dma_start_transpose(
                            out=xtv_b, in_=xsb[:]
                        )
                        nc.gpsimd.dma_start(out=xt8[b][:], in_=xt[b][:])
                    else:
                        for k in range(2):
                            nc.sync.dma_start_transpose(
                                out=xtv_b[:, k * NB + t0:k * NB + t0 + cn, :],
                                in_=xsb[:, (k * NB + t0) * 128:
                                        (k * NB + t0 + cn) * 128],
                            )
                            nc.gpsimd.dma_start(
                                out=xt8v_b[:, k, t0 * 128:(t0 + cn) * 128],
                                in_=xtbv_b[:, k, t0 * 128:(t0 + cn) * 128],
                            )

            def emit_phase2(g):
                for it in range(ITILES):
                    pg = dpsum.tile([128, GW], f32, tag="pg")
                    lhs = xt8v[0][:, :, it * 128:(it + 1) * 128]
                    for c4 in range(4):
                        rhs = xt8v[g][:, :, c4 * 512:(c4 + 1) * 512]
                        nc.tensor.matmul(
                            pg[:, c4 * 512:(c4 + 1) * 512], lhs, rhs,
                            start=True, stop=True,
                            perf_mode=mybir.MatmulPerfMode.DoubleRow,
                        )
                    if g == 0:
                        db = 128 * it
                        nc.vector.tensor_add(
                            pg[:, db:db + 128], pg[:, db:db + 128], mneg[:]
                        )
                    slot = it * NGROUPS + g
                    if (g * ITILES + it) % 5 in (0, 2):
                        nc.vector.reduce_max(
                            md[:, slot:slot + 1], pg[:],
                            axis=mybir.AxisListType.X,
                        )
                    else:
                        esc = esc_pool.tile([128, GW], bf16, tag="esc")
                        nc.scalar.activation(
                            esc[:], pg[:], AF.Exp,
                            scale=BETA, bias=ebias[:],
                            accum_out=ms[:, slot:slot + 1],
                        )

            # interleave: keep phase 1 two batches ahead of phase 2
            for b in range(NGROUPS):
                emit_phase1(b)
                if b >= 2:
                    emit_phase2(b - 2)
            emit_phase2(NGROUPS - 2)
            emit_phase2(NGROUPS - 1)

            nc.sync.dma_start(out=md_out, in_=md[:])
            nc.sync.dma_start(out=ms_out, in_=ms[:])

    nc.compile()
    return nc


def _get_nc():
    if "nc" not in _CACHE:
        _CACHE["nc"] = _build()
    return _CACHE["nc"]


def kernel(student_output: np.ndarray) -> np.ndarray:
    s = np.ascontiguousarray(np.asarray(student_output, dtype=np.float32))
    assert s.shape == (N, D)

    nc = _get_nc()
    in_maps = [
        {"x": np.ascontiguousarray(
            np.roll(s, -c * ROWS, axis=0).reshape(NGROUPS, NB, 128, D))}
        for c in range(NCORES)
    ]
    import os
    kwargs = {}
    if os.environ.get("KOLEO_TRACE"):
        kwargs = {"trace": True, "tmpdir": os.environ.get("KOLEO_TRACE_DIR") or None}
    res = bass_utils.run_bass_kernel_spmd(
        nc, in_maps, core_ids=list(range(NCORES)), **kwargs
    )
    _CACHE["last_results"] = res

    m_parts = []
    for c in range(NCORES):
        md = res.results[c]["md_out"].astype(np.float64)   # [128, 16*8]
        ms = res.results[c]["ms_out"].astype(np.float64)
        md = md.reshape(128, ITILES, NGROUPS)
        ms = ms.reshape(128, ITILES, NGROUPS)
        m_dve = md.max(axis=2)                             # [128, it]
        s_sum = ms.sum(axis=2)                             # [128, it]
        with np.errstate(divide="ignore"):
            m_lse = SHIFT + np.log(s_sum) / BETA
        m_loc = np.maximum(m_dve, m_lse)                   # [p, it]
        m_parts.append(m_loc.T.reshape(ROWS))              # local row = it*128+p
    m = np.concatenate(m_parts)

    d2 = np.maximum(2.0 - 2.0 * m, 0.0)
    loss = -np.mean(np.log(np.sqrt(d2) + EPS))
    return np.array(loss, dtype=np.float32)


# revision 51
# speedup vs baseline: 1.0237x; 1.0237x over previous
"""KoLeo loss kernel for Trainium2 (8 NeuronCores).

Computes -mean(log(||x_i - x_{nn(i)} + eps||)) where x = row-normalized
student_output and nn(i) is the nearest neighbor by max inner product
(diagonal excluded). For unit vectors ||x_i - x_j||^2 = 2 - 2*<x_i,x_j>,
so only the per-row max off-diagonal inner product m_i is needed. Each
core handles a 2048-row block (input rolled so its rows are local 0..2047).

Design (v3):
  - Phase 1 (per 2048-row batch): DMA row tiles, batched squares on ACT +
    one 3D DVE reduce for norms^2, all-DVE rsqrt (quake guess + 2 Newton),
    scale rows to unit length in bf16 with the two 128-wide d-halves split
    (DVE), one DMA-XBAR transpose (sync queue) -> k-major transposed tile,
    then GpSimd casts bf16 -> fp8e4 (max(a,a) tensor_tensor, the one
    elementwise op the Pool engine supports).
  - Phase 2 (per j-group, emitted interleaved two batches behind phase 1):
    fp8 DoubleRow matmuls (K=256 in one pass, N=512 chunks), so half the
    matmul instructions of a bf16 version at the same ~379ns/MM stream
    rate this silicon sustains.
  - Per-unit [128,2048] PSUM drain split ~50/50 between the only two
    PSUM-capable engines: DVE exact reduce_max; ACT exp-accumulate
    (log-sum-exp upper bound of max, beta=256 shift=0.45, adds ~1e-3
    relative loss error, validated offline). Diagonal killed by a -3 mask
    add before either consumer.
  - Host: m = max(dve_max, shift + log(sum exp)/beta) per row, then loss.
    fp8e4 quantization of unit rows adds ~2e-3 relative loss error
    (validated offline; gate is 2e-2).
"""

import numpy as np

import concourse.bass as bass
import concourse.mybir as mybir
import concourse.tile as tile
from concourse import bacc
from concourse import bass_utils

N = 16384
D = 256
NCORES = 8
ROWS = N // NCORES          # 2048 rows per core
ITILES = ROWS // 128        # 16 i-tiles per core
NT = N // 128               # 128 row-tiles of the full matrix
GW = 2048                   # j-group width (4 PSUM banks of fp32)
NGROUPS = N // GW           # 8 j-groups
NB = 16                     # row-tiles per batch (= one j-group)
EPS = 1e-8
BETA = 256.0
SHIFT = 0.45

_CACHE = {}


def _build():
    f32 = mybir.dt.float32
    bf16 = mybir.dt.bfloat16
    fp8 = mybir.dt.float8e4
    i32 = mybir.dt.int32
    AF = mybir.ActivationFunctionType
    ALU = mybir.AluOpType

    nc = bacc.Bacc("TRN2", target_bir_lowering=False, debug=False)
    x = nc.dram_tensor("x", [NGROUPS, NB, 128, D], f32, kind="ExternalInput").ap()
    md_out = nc.dram_tensor("md_out", [128, ITILES * NGROUPS], f32,
                            kind="ExternalOutput").ap()
    ms_out = nc.dram_tensor("ms_out", [128, ITILES * NGROUPS], f32,
                            kind="ExternalOutput").ap()

    with tile.TileContext(nc) as tc:
        with (
            tc.tile_pool(name="singles", bufs=1) as singles,
            tc.tile_pool(name="sbig", bufs=2) as sbig_pool,
            tc.tile_pool(name="sq", bufs=2) as sq_pool,
            tc.tile_pool(name="xsb", bufs=2) as xsb_pool,
            tc.tile_pool(name="xt", bufs=1) as xt_pool,
            tc.tile_pool(name="small", bufs=6) as small,
            tc.tile_pool(name="esc", bufs=2) as esc_pool,
            tc.tile_pool(name="dpsum", bufs=2, space="PSUM") as dpsum,
        ):
            # Diagonal knock-out mask: -3 on the diagonal of a 128x128 block.
            mneg = singles.tile([128, 128], f32, tag="mneg")
            nc.gpsimd.memset(mneg[:], 0.0)
            nc.gpsimd.affine_select(
                out=mneg[:], in_=mneg[:], compare_op=ALU.not_equal,
                fill=-3.0, base=0, pattern=[[-1, 128]], channel_multiplier=1,
            )

            ebias = singles.tile([128, 1], f32, tag="ebias")
            nc.vector.memset(ebias[:], -BETA * SHIFT)

            ss = singles.tile([128, NT], f32, tag="ss")      # norms^2, [p, tile]
            md = singles.tile([128, ITILES * NGROUPS], f32, tag="md")
            ms = singles.tile([128, ITILES * NGROUPS], f32, tag="ms")
            nc.vector.memset(md[:], -3.0)
            nc.vector.memset(ms[:], 0.0)

            # k-major transposed normalized tiles per j-group:
            # xt[g][dd, k*2048 + j] = xnorm[g*2048 + j, k*128 + dd]
            xt = [
                xt_pool.tile([128, 2 * GW], bf16, tag=f"xt{g}", name=f"xt{g}")
                for g in range(NGROUPS)
            ]
            xt8 = [
                xt_pool.tile([128, 2 * GW], fp8, tag=f"xt8_{g}", name=f"xt8_{g}")
                for g in range(NGROUPS)
            ]
            xt8v = [
                xt8[g][:].rearrange("p (k j) -> p k j", k=2)
                for g in range(NGROUPS)
            ]

            # PE warm-up: 32 junk matmuls so the HAM clock-gate is open
            # before the real dot stream arrives.
            wj = singles.tile([128, 1024], bf16, tag="wj")
            nc.vector.memset(wj[:], 0.0)
            wp = dpsum.tile([128, GW], f32, tag="pg")
            for _ in range(32):
                nc.tensor.matmul(
                    wp[:, 0:512], wj[:, 0:128], wj[:, 512:1024],
                    start=True, stop=True,
                )

            def emit_phase1_a(b):
                # DMA in + norms^2 + rsqrt
                sb = sbig_pool.tile([128, NB, D], f32, tag="sb")
                nc.sync.dma_start(
                    out=sb[:], in_=x[b].rearrange("t p d -> p t d")
                )
                sqb = sq_pool.tile([128, NB * D], f32, tag="sqb")
                sb_flat = sb[:].rearrange("p t d -> p (t d)")
                nc.scalar.activation(sqb[:], sb_flat, AF.Square)
                ssb = ss[:, b * NB:(b + 1) * NB]
                nc.vector.tensor_reduce(
                    ssb, sqb[:].rearrange("p (t d) -> p t d", d=D),
                    axis=mybir.AxisListType.X, op=ALU.add,
                )
                # r = rsqrt(ss): quake initial guess + 2 Newton steps, all DVE
                sh = small.tile([128, NB], i32, tag="sh")
                nc.vector.tensor_scalar(
                    out=sh[:], in0=ssb.bitcast(i32), scalar1=1, scalar2=None,
                    op0=ALU.logical_shift_right,
                )
                r0 = small.tile([128, NB], i32, tag="r0")
                nc.vector.tensor_scalar(
                    out=r0[:], in0=sh[:], scalar1=-1, scalar2=0x5F3759DF,
                    op0=ALU.mult, op1=ALU.add,
                )
                r = r0[:].bitcast(f32)
                for _ in range(2):
                    t1 = small.tile([128, NB], f32, tag="t1")
                    nc.vector.tensor_mul(t1[:], r, r)
                    nc.vector.tensor_mul(t1[:], t1[:], ssb)
                    nc.vector.tensor_scalar(
                        out=t1[:], in0=t1[:], scalar1=-0.5, scalar2=1.5,
                        op0=ALU.mult, op1=ALU.add,
                    )
                    r2 = small.tile([128, NB], f32, tag="rr")
                    nc.vector.tensor_mul(r2[:], r, t1[:])
                    r = r2[:]
                return sb, r

            def emit_phase1_b(b, sb, r):
                # scale rows to unit norm -> bf16 (k-major), transpose, cast
                xsb = xsb_pool.tile([128, 2 * NB * 128], bf16, tag="xsb")
                xsbv = xsb[:].rearrange("p (k t c) -> p k t c", k=2, c=128)
                for t in range(NB):
                    nc.vector.tensor_scalar_mul(
                        xsbv[:, :, t, :],
                        sb[:, t, :].rearrange("p (k c) -> p k c", k=2),
                        r[:, t:t + 1],
                    )
                nc.sync.dma_start_transpose(
                    out=xt[b][:].rearrange("p (s c) -> p s c", c=128),
                    in_=xsb[:],
                )
                nc.gpsimd.dma_start(out=xt8[b][:], in_=xt[b][:])

            def emit_phase2(g, its=range(ITILES)):
                for it in its:
                    pg = dpsum.tile([128, GW], f32, tag="pg")
                    lhs = xt8v[0][:, :, it * 128:(it + 1) * 128]
                    for c4 in range(4):
                        rhs = xt8v[g][:, :, c4 * 512:(c4 + 1) * 512]
                        nc.tensor.matmul(
                            pg[:, c4 * 512:(c4 + 1) * 512], lhs, rhs,
                            start=True, stop=True,
                            perf_mode=mybir.MatmulPerfMode.DoubleRow,
                        )
                    if g == 0:
                        db = 128 * it
                        nc.vector.tensor_add(
                            pg[:, db:db + 128], pg[:, db:db + 128], mneg[:]
                        )
                    slot = it * NGROUPS + g
                    if (g * ITILES + it) % 5 in (0, 2):
                        nc.vector.reduce_max(
                            md[:, slot:slot + 1], pg[:],
                            axis=mybir.AxisListType.X,
                        )
                    else:
                        esc = esc_pool.tile([128, GW], bf16, tag="esc")
                        nc.scalar.activation(
                            esc[:], pg[:], AF.Exp,
                            scale=BETA, bias=ebias[:],
                            accum_out=ms[:, slot:slot + 1],
                        )

            # interleave: phase 1 two batches ahead, with each group's units
            # emitted in halves around phase-1 chunks so DVE/ACT alternate
            # between producing operands and draining PSUM
            for b in range(NGROUPS):
                sb, r = emit_phase1_a(b)
                if b >= 2:
                    emit_phase2(b - 2, range(0, 8))
                emit_phase1_b(b, sb, r)
                if b >= 2:
                    emit_phase2(b - 2, range(8, 16))
            emit_phase2(NGROUPS - 2)
            emit_phase2(NGROUPS - 1)

            nc.sync.dma_start(out=md_out, in_=md[:])
            nc.sync.dma_start(out=ms_out, in_=ms[:])

    nc.compile()
    return nc


def _get_nc():
    if "nc" not in _CACHE:
        _CACHE["nc"] = _build()
    return _CACHE["nc"]


def kernel(student_output: np.ndarray) -> np.ndarray:
    s = np.ascontiguousarray(np.asarray(student_output, dtype=np.float32))
    assert s.shape == (N, D)

    nc = _get_nc()
    in_maps = [
        {"x": np.ascontiguousarray(
            np.roll(s, -c * ROWS, axis=0).reshape(NGROUPS, NB, 128, D))}
        for c in range(NCORES)
    ]
    import os
    kwargs = {}
    if os.environ.get("KOLEO_TRACE"):
        kwargs = {"trace": True, "tmpdir": os.environ.get("KOLEO_TRACE_DIR") or None}
    res = bass_utils.run_bass_kernel_spmd(
        nc, in_maps, core_ids=list(range(NCORES)), **kwargs
    )
    _CACHE["last_results"] = res

    m_parts = []
    for c in range(NCORES):
        md = res.results[c]["md_out"].astype(np.float64)   # [128, 16*8]
        ms = res.results[c]["ms_out"].astype(np.float64)
        md = md.reshape(128, ITILES, NGROUPS)
        ms = ms.reshape(128, ITILES, NGROUPS)
        m_dve = md.max(axis=2)                             # [128, it]
        s_sum = ms.sum(axis=2)                             # [128, it]
        with np.errstate(divide="ignore"):
            m_lse = SHIFT + np.log(s_sum) / BETA
        m_loc = np.maximum(m_dve, m_lse)                   # [p, it]
        m_parts.append(m_loc.T.reshape(ROWS))              # local row = it*128+p
    m = np.concatenate(m_parts)

    d2 = np.maximum(2.0 - 2.0 * m, 0.0)
    loss = -np.mean(np.log(np.sqrt(d2) + EPS))
    return np.array(loss, dtype=np.float32)
